# revision 1
# baseline (speedup 1.0000x reference)
"""2-layer LSTM (B=64, T=512, in=256, hidden=1024) on 8 trn2 NeuronCores.

Strategy: tensor-parallel over the 4m=4096 gate dimension (512 gate cols per
core, grouped [i|f|o|g] x 128 so one sigmoid covers 384 cols), batch whole.
Layer 1 runs one step behind layer 0 so a single fused AllGather per timestep
exchanges the transposed h-slices {h0_s, h1_(s-1)}.  The x@W_ih0+b0 projection
for all timesteps is precomputed on-device as one batched matmul (phase A);
layer 1's h0@W_ih1 is folded into its recurrent PSUM accumulation.  All
weights stay SBUF-resident.  Self-contained: only needs concourse on the
PYTHONPATH (as provided in the container).
"""
import numpy as np
from contextlib import ExitStack

from concourse import bacc, tile, mybir
from concourse.bass_utils import run_bass_kernel_spmd

F32 = mybir.dt.float32
R = 8            # cores
B = 64           # batch
N_IN = 256       # layer-0 input width
M = 1024         # hidden
T = 512          # sequence length
S = 4 * M // R   # gate cols per rank (512)
MS = M // R      # m cols per rank (128)
KT = M // 128    # k-tiles over hidden (8)

AF = mybir.ActivationFunctionType

_CACHE = {}


def _gate_cols(j):
    # [i | f | o | g] so sigmoid covers cols 0:384, tanh 384:512.
    # (z layout in the reference is i, f, g, o blocks of M)
    idx = []
    for gblk in (0, 1, 3, 2):
        idx.extend(range(gblk * M + j * MS, gblk * M + (j + 1) * MS))
    return np.array(idx)


def _build_kernel():
    nc = bacc.Bacc("TRN2", target_bir_lowering=False, debug=False, num_devices=R)

    P = lambda name, shape: nc.declare_dram_parameter(name, shape, F32, isOutput=False)
    d_xT = P("xT", [N_IN, T * B])
    d_Wih0 = P("Wih0", [N_IN, S]); d_b0row = P("b0row", [1, S])
    d_Whh0 = P("Whh0", [M, S])
    d_Wih1 = P("Wih1", [M, S]); d_Whh1 = P("Whh1", [M, S]); d_b1row = P("b1row", [1, S])
    d_h0T = P("h0T", [M, B]); d_h1T = P("h1T", [M, B]); d_h1Tm = P("h1Tm", [MS, B])
    d_c0 = P("c0", [B, MS]); d_c1 = P("c1", [B, MS])
    d_ident = P("ident", [B, B]); d_ones = P("ones", [1, 128])

    d_out = nc.declare_dram_parameter("out_h1", [T * B, MS], F32, isOutput=True)
    d_hf = nc.declare_dram_parameter("hf", [2 * B, MS], F32, isOutput=True)
    d_cf = nc.declare_dram_parameter("cf", [2 * B, MS], F32, isOutput=True)

    d_Z0 = nc.dram_tensor("Z0scratch", [T * B, S], F32)
    ag_in = [nc.dram_tensor(f"ag_in{p}", [2 * MS, B], F32) for p in range(2)]
    ag_out = [nc.dram_tensor(f"ag_out{p}", [R * 2 * MS, B], F32, addr_space="Shared")
              for p in range(2)]
    rg = [list(range(R))]

    with tile.TileContext(nc) as tc, ExitStack() as ctx:
        wpool = ctx.enter_context(tc.tile_pool(name="w", bufs=1))
        state = ctx.enter_context(tc.tile_pool(name="state", bufs=1))
        pool = ctx.enter_context(tc.tile_pool(name="work", bufs=3))
        zpool = ctx.enter_context(tc.tile_pool(name="z0", bufs=3))
        psum = ctx.enter_context(tc.tile_pool(name="ps", bufs=2, space="PSUM"))
        xpool = ctx.enter_context(tc.tile_pool(name="xp", bufs=3))

        wWhh0 = wpool.tile([128, KT, S], F32)
        wWih1 = wpool.tile([128, KT, S], F32)
        wWhh1 = wpool.tile([128, KT, S], F32)
        wWih0 = wpool.tile([128, 2, S], F32)
        nc.sync.dma_start(wWhh0[:], d_Whh0.ap().rearrange("(k p) s -> p k s", p=128))
        nc.sync.dma_start(wWih1[:], d_Wih1.ap().rearrange("(k p) s -> p k s", p=128))
        nc.sync.dma_start(wWhh1[:], d_Whh1.ap().rearrange("(k p) s -> p k s", p=128))
        nc.sync.dma_start(wWih0[:], d_Wih0.ap().rearrange("(k p) s -> p k s", p=128))
        b0row = wpool.tile([1, S], F32); b1row = wpool.tile([1, S], F32)
        ident = wpool.tile([B, B], F32); ones = wpool.tile([1, 128], F32)
        nc.sync.dma_start(b0row[:], d_b0row[:])
        nc.sync.dma_start(b1row[:], d_b1row[:])
        nc.sync.dma_start(ident[:], d_ident[:])
        nc.sync.dma_start(ones[:], d_ones[:])

        h0T = state.tile([128, KT, B], F32)
        h1T = state.tile([128, KT, B], F32)
        c0 = state.tile([B, MS], F32)
        c1 = state.tile([B, MS], F32)
        nc.sync.dma_start(h0T[:], d_h0T.ap().rearrange("(k p) b -> p k b", p=128))
        nc.sync.dma_start(h1T[:], d_h1T.ap().rearrange("(k p) b -> p k b", p=128))
        nc.sync.dma_start(c0[:], d_c0[:])
        nc.sync.dma_start(c1[:], d_c1[:])

        # phase A: Z0 rows = [x_row, 1] @ [Wih0; b0]
        for mt in range(T * B // 128):
            xt = xpool.tile([128, 2, 128], F32, tag="xt")
            nc.sync.dma_start(
                xt[:],
                d_xT.ap()[:, mt * 128:(mt + 1) * 128].rearrange("(k p) c -> p k c", p=128))
            zp = psum.tile([128, S], F32, tag="z0p")
            nc.tensor.matmul(zp[:], xt[:, 0, :], wWih0[:, 0, :], start=True, stop=False)
            nc.tensor.matmul(zp[:], xt[:, 1, :], wWih0[:, 1, :], start=False, stop=False)
            nc.tensor.matmul(zp[:], ones[:, :], b0row[:], start=False, stop=True)
            zs = xpool.tile([128, S], F32, tag="zsb")
            nc.vector.tensor_copy(zs[:], zp[:])
            nc.sync.dma_start(d_Z0[mt * 128:(mt + 1) * 128, :], zs[:])

        def lstm_tail(zc, cstate):
            sig = pool.tile([B, 3 * MS], F32, tag="sig")
            tg = pool.tile([B, MS], F32, tag="tg")
            nc.scalar.activation(sig[:], zc[:, 0:3 * MS], AF.Sigmoid)
            nc.scalar.activation(tg[:], zc[:, 3 * MS:S], AF.Tanh)
            fc = pool.tile([B, MS], F32, tag="fc")
            nc.vector.tensor_mul(fc[:], sig[:, MS:2 * MS], cstate[:])
            ig = pool.tile([B, MS], F32, tag="ig")
            nc.vector.tensor_mul(ig[:], sig[:, 0:MS], tg[:])
            nc.vector.tensor_add(cstate[:], fc[:], ig[:])
            tc_ = pool.tile([B, MS], F32, tag="tc")
            nc.scalar.activation(tc_[:], cstate[:], AF.Tanh)
            h = pool.tile([B, MS], F32, tag="h")
            nc.vector.tensor_mul(h[:], sig[:, 2 * MS:3 * MS], tc_[:])
            return h

        def send_transposed(h, p, half):
            tp = psum.tile([MS, B], F32, tag="tpp")
            nc.tensor.transpose(tp[:], h[:], ident[:])
            hm = pool.tile([MS, B], F32, tag=f"hTm{half}")
            nc.vector.tensor_copy(hm[:], tp[:])
            nc.sync.dma_start(ag_in[p][half * MS:(half + 1) * MS, :], hm[:])

        def layer0(t):
            z0p = psum.tile([B, S], F32, tag="z0p")
            z0t = zpool.tile([B, S], F32, tag="z0t")
            nc.sync.dma_start(z0t[:], d_Z0[t * B:(t + 1) * B, :])
            for k in range(KT):
                nc.tensor.matmul(z0p[:], h0T[:, k, :], wWhh0[:, k, :],
                                 start=(k == 0), stop=False)
            nc.tensor.matmul(z0p[:], ident[:], z0t[:], start=False, stop=True)
            return lstm_tail(z0p, c0)

        def layer1(t):
            z1p = psum.tile([B, S], F32, tag="z1p")
            for k in range(KT):
                nc.tensor.matmul(z1p[:], h0T[:, k, :], wWih1[:, k, :],
                                 start=(k == 0), stop=False)
            for k in range(KT):
                nc.tensor.matmul(z1p[:], h1T[:, k, :], wWhh1[:, k, :],
                                 start=False, stop=False)
            nc.tensor.matmul(z1p[:], ones[:, 0:B], b1row[:], start=False, stop=True)
            h1 = lstm_tail(z1p, c1)
            nc.sync.dma_start(d_out[t * B:(t + 1) * B, :], h1[:])
            return h1

        def gather(p):
            nc.gpsimd.collective_compute(
                "AllGather", mybir.AluOpType.bypass, replica_groups=rg,
                ins=[ag_in[p][:]], outs=[ag_out[p][:]])

        def readback(p):
            src = ag_out[p].ap().rearrange("(r l p) b -> l p r b", r=R, l=2)
            nc.sync.dma_start(h0T[:], src[0])
            nc.sync.dma_start(h1T[:], src[1])

        h0 = layer0(0)
        send_transposed(h0, 0, 0)
        h1m0 = pool.tile([MS, B], F32, tag="hTm1")
        nc.sync.dma_start(h1m0[:], d_h1Tm[:])
        nc.sync.dma_start(ag_in[0][MS:2 * MS, :], h1m0[:])
        gather(0)

        for s in range(1, T):
            p_prev, p = (s - 1) % 2, s % 2
            readback(p_prev)
            h0 = layer0(s)
            send_transposed(h0, p, 0)
            h1 = layer1(s - 1)
            send_transposed(h1, p, 1)
            gather(p)

        readback((T - 1) % 2)
        h1 = layer1(T - 1)

        hf = pool.tile([B, 2, MS], F32, tag="hfin")
        nc.vector.tensor_copy(hf[:, 0, :], h0[:])
        nc.vector.tensor_copy(hf[:, 1, :], h1[:])
        nc.sync.dma_start(d_hf.ap().rearrange("(l b) s -> b l s", l=2), hf[:])
        cf = pool.tile([B, 2, MS], F32, tag="cfin")
        nc.vector.tensor_copy(cf[:, 0, :], c0[:])
        nc.vector.tensor_copy(cf[:, 1, :], c1[:])
        nc.sync.dma_start(d_cf.ap().rearrange("(l b) s -> b l s", l=2), cf[:])

    nc.compile()
    return nc


def _host_prep(x, h, c, W_ih0, W_hh0, b0, W_ih1, W_hh1, b1):
    xr = np.ascontiguousarray(np.transpose(np.asarray(x), (1, 0, 2)).reshape(T * B, N_IN))
    xT = np.ascontiguousarray(xr.T).astype(np.float32)
    h = np.asarray(h); c = np.asarray(c)
    in_maps = []
    for j in range(R):
        cols = _gate_cols(j)
        sl = slice(j * MS, (j + 1) * MS)
        im = {
            "xT": xT,
            "Wih0": np.asarray(W_ih0)[:, cols],
            "b0row": np.asarray(b0)[cols][None, :],
            "Whh0": np.asarray(W_hh0)[:, cols],
            "Wih1": np.asarray(W_ih1)[:, cols],
            "Whh1": np.asarray(W_hh1)[:, cols],
            "b1row": np.asarray(b1)[cols][None, :],
            "h0T": h[0].T,
            "h1T": h[1].T,
            "h1Tm": h[1].T[sl],
            "c0": c[0][:, sl],
            "c1": c[1][:, sl],
            "ident": np.eye(B, dtype=np.float32),
            "ones": np.ones((1, 128), dtype=np.float32),
        }
        in_maps.append({k: np.ascontiguousarray(v, dtype=np.float32)
                        for k, v in im.items()})
    return in_maps


def kernel(x, h, c, W_ih0, W_hh0, b0, W_ih1, W_hh1, b1):
    if "nc" not in _CACHE:
        _CACHE["nc"] = _build_kernel()
    nc = _CACHE["nc"]
    in_maps = _host_prep(x, h, c, W_ih0, W_hh0, b0, W_ih1, W_hh1, b1)
    res = run_bass_kernel_spmd(nc, in_maps, list(range(R))).results

    outs = np.empty((B, T, M), np.float32)
    h_f = np.empty((2, B, M), np.float32)
    c_f = np.empty((2, B, M), np.float32)
    for j in range(R):
        r = res[j]
        sl = slice(j * MS, (j + 1) * MS)
        outs[:, :, sl] = np.transpose(r["out_h1"].reshape(T, B, MS), (1, 0, 2))
        h_f[:, :, sl] = r["hf"].reshape(2, B, MS)
        c_f[:, :, sl] = r["cf"].reshape(2, B, MS)
    return outs, h_f, c_f
